# revision 7
# baseline (speedup 1.0000x reference)
"""ALISTA (nn_ALISTA) Trainium2 kernel — data-parallel over batch on 8 NeuronCores.

Reference computation (per iteration i, 16 iterations):
    r   = d @ A.T - y                      # [B, m]
    z   = d - step_i * (r @ W)             # [B, n]
    d'  = sign(z) * max(|z| - thr_i, 0)    # soft threshold
output = all 16 iterates stacked: [16, B, n].

Shapes: B=4096, m=512, n=2048. Sharding: batch/8 -> 512 rows per core;
A, W, thr, step replicated. No cross-core communication.

Device layout (per core) keeps d TRANSPOSED as dT [n, b] so that both
matmuls need zero on-chip transposes:
  step 1: rT[m,b] = sum_n AT[n,m] * dT[n,b]   (lhsT = A.T tiles, rhs = dT)
  step 2: q[n,b]  = sum_m W[m,n] * rs[m,b]    (lhsT = W as-is,   rhs = rT - yT)
  z = dT - step*q computed as one scalar_tensor_tensor; soft-threshold via a
  ReLU pair on the Scalar engine: soft(z,t) = relu(z-t) - relu(-z-t).
Host pre-transposes A and y (cheap numpy) and un-transposes the output.
"""

import os

import numpy as np

M, N, ITERS = 512, 2048, 16
B_FULL = 4096
NCORES = 8
BL = B_FULL // NCORES  # 512 rows of y per core
KT = N // 128  # 16 n-tiles (contraction tiles for step 1 / output tiles for step 2)
MT = M // 128  # 4 m-tiles

# Matmul compute dtype: float32r streams 1 row/cycle (like bf16) vs float32's
# 4 rows/cycle, at reduced internal precision. Storage stays float32 either way.
MM_DTYPE = os.environ.get("ALISTA_MM_DTYPE", "float32r")

_CACHE = {}
LAST = {}  # exec_time_ns etc. from the most recent kernel() call


def build_nc(mm_dtype_str=MM_DTYPE):
    import concourse.bass as bass
    from concourse import bacc
    import concourse.mybir as mybir
    import concourse.tile as tile
    from contextlib import ExitStack

    f32 = mybir.dt.float32
    mmdt = getattr(mybir.dt, mm_dtype_str)
    Relu = mybir.ActivationFunctionType.Relu
    Alu = mybir.AluOpType

    nc = bacc.Bacc("TRN2", target_bir_lowering=False, debug=False, num_devices=NCORES)

    at_ext = nc.dram_tensor("at", [128, KT * M], f32, kind="ExternalInput").ap()
    w_ext = nc.dram_tensor("w", [128, MT * N], f32, kind="ExternalInput").ap()
    yt_ext = nc.dram_tensor("yt", [128, MT * BL], f32, kind="ExternalInput").ap()
    cst_ext = nc.dram_tensor("cst", [128, 2 * ITERS], f32, kind="ExternalInput").ap()
    out_ext = nc.dram_tensor("out", [ITERS, N, BL], f32, kind="ExternalOutput").ap()

    with tile.TileContext(nc) as tc, ExitStack() as ctx:
        const = ctx.enter_context(tc.tile_pool(name="const", bufs=1))
        dpool = ctx.enter_context(tc.tile_pool(name="d", bufs=1))
        rspool = ctx.enter_context(tc.tile_pool(name="rs", bufs=2))
        upool = ctx.enter_context(tc.tile_pool(name="u", bufs=3))
        apool = ctx.enter_context(tc.tile_pool(name="act", bufs=3))
        prpool = ctx.enter_context(tc.tile_pool(name="pr", bufs=1, space="PSUM"))
        pzpool = ctx.enter_context(tc.tile_pool(name="pz", bufs=4, space="PSUM"))

        at_sb = const.tile([128, KT * M], mmdt, name="at_sb", tag="at")
        w_sb = const.tile([128, MT * N], mmdt, name="w_sb", tag="w")
        yt_sb = const.tile([128, MT * BL], f32, name="yt_sb", tag="yt")
        cst_sb = const.tile([128, 2 * ITERS], f32, name="cst_sb", tag="cst")
        nc.sync.dma_start(at_sb[:], at_ext[:].bitcast(mmdt))
        nc.sync.dma_start(w_sb[:], w_ext[:].bitcast(mmdt))
        nc.sync.dma_start(yt_sb[:], yt_ext[:])
        nc.sync.dma_start(cst_sb[:], cst_ext[:])

        # Two d buffers (ping/pong across iterations), 16 tiles each so the
        # scheduler tracks per-tile deps and can overlap iteration boundaries.
        d_sb = [
            [dpool.tile([128, BL], mmdt, name=f"d{p}_{k}", tag=f"d{p}_{k}") for k in range(KT)]
            for p in range(2)
        ]

        for it in range(ITERS):
            negthr = cst_sb[:, 2 * it : 2 * it + 1]
            negstep = cst_sb[:, 2 * it + 1 : 2 * it + 2]
            dr = d_sb[it % 2]  # previous iterate (unused when it == 0: d0 = 0)
            dw = d_sb[(it + 1) % 2]

            # ---- step 1: rs[m] = rT_tile - yT_tile  (r = d @ A.T - y) ----
            rs = [rspool.tile([128, BL], mmdt, name=f"rs_{it}_{m}", tag=f"rs{m}") for m in range(MT)]
            if it == 0:
                for m in range(MT):
                    nc.vector.tensor_scalar_mul(
                        rs[m][:], yt_sb[:, m * BL : (m + 1) * BL], -1.0
                    )
            else:
                for m in range(MT):
                    prt = prpool.tile([128, BL], f32, name=f"pr_{it}_{m}", tag=f"pr{m}")
                    for k in range(KT):
                        nc.tensor.matmul(
                            prt[:],
                            at_sb[:, k * M + m * 128 : k * M + (m + 1) * 128],
                            dr[k][:],
                            start=(k == 0),
                            stop=(k == KT - 1),
                        )
                    nc.vector.tensor_sub(
                        rs[m][:], prt[:], yt_sb[:, m * BL : (m + 1) * BL]
                    )

            # ---- step 2 + soft threshold, per output n-tile ----
            for n in range(KT):
                pzt = pzpool.tile([128, BL], f32, name=f"pz_{it}_{n}", tag="pz")
                for m in range(MT):
                    nc.tensor.matmul(
                        pzt[:],
                        w_sb[:, m * N + n * 128 : m * N + (n + 1) * 128],
                        rs[m][:],
                        start=(m == 0),
                        stop=(m == MT - 1),
                    )
                # z = d - step*q  == (q * (-step)) + d
                u = upool.tile([128, BL], f32, name=f"u_{it}_{n}", tag="u")
                if it == 0:
                    nc.vector.tensor_scalar_mul(u[:], pzt[:], negstep)
                else:
                    nc.vector.scalar_tensor_tensor(
                        u[:], pzt[:], negstep, dr[n][:].bitcast(f32), op0=Alu.mult, op1=Alu.add
                    )
                # soft(z, t) = relu(z - t) - relu(-z - t)
                a1 = apool.tile([128, BL], f32, name=f"a1_{it}_{n}", tag="a1")
                a2 = apool.tile([128, BL], f32, name=f"a2_{it}_{n}", tag="a2")
                nc.scalar.activation(a1[:], u[:], Relu, bias=negthr, scale=1.0)
                nc.scalar.activation(a2[:], u[:], Relu, bias=negthr, scale=-1.0)
                nc.vector.tensor_sub(dw[n][:], a1[:], a2[:])
                nc.sync.dma_start(out_ext[it, n * 128 : (n + 1) * 128, :], dw[n][:].bitcast(f32))

    nc.compile()
    return nc


def _get_nc():
    key = MM_DTYPE
    if key not in _CACHE:
        _CACHE[key] = build_nc(key)
    return _CACHE[key]


def make_in_maps(y, A, W, thr, step):
    y = np.asarray(y, dtype=np.float32)
    A = np.asarray(A, dtype=np.float32)
    W = np.asarray(W, dtype=np.float32)
    thr = np.asarray(thr, dtype=np.float32)
    step = np.asarray(step, dtype=np.float32)

    # [n, m] -> SBUF layout [p=128, k*M + m] with row p holding A.T[k*128+p, :]
    at_h = np.ascontiguousarray(
        A.T.reshape(KT, 128, M).transpose(1, 0, 2).reshape(128, KT * M)
    )
    w_h = np.ascontiguousarray(
        W.reshape(MT, 128, N).transpose(1, 0, 2).reshape(128, MT * N)
    )
    cst = np.zeros((128, 2 * ITERS), np.float32)
    cst[:, 0::2] = -thr[None, :]
    cst[:, 1::2] = -step[None, :]

    yT = y.T  # [m, B]
    in_maps = []
    for c in range(NCORES):
        ytc = np.ascontiguousarray(
            yT[:, c * BL : (c + 1) * BL]
            .reshape(MT, 128, BL)
            .transpose(1, 0, 2)
            .reshape(128, MT * BL)
        )
        in_maps.append({"at": at_h, "w": w_h, "yt": ytc, "cst": cst})
    return in_maps


def kernel(y, A, W, thr, step):
    from concourse.bass_utils import run_bass_kernel_spmd

    nc = _get_nc()
    in_maps = make_in_maps(y, A, W, thr, step)

    res = run_bass_kernel_spmd(nc, in_maps, list(range(NCORES)))
    LAST["exec_time_ns"] = res.exec_time_ns
    results = res.results

    # per-core out: [16, n, b_local] -> full [16, B, n]
    out = np.concatenate([r["out"].transpose(0, 2, 1) for r in results], axis=1)
    return np.ascontiguousarray(out, dtype=np.float32)


def make_exec_fn(nc, in_maps):
    """Build a re-executable jitted fn over the 8-core mesh (no donation, so
    it can be called repeatedly on resident device buffers) for timing.
    Mirrors bass2jax.run_bass_via_pjrt's multi-core path."""
    import jax
    import numpy as _np
    from jax.sharding import Mesh, PartitionSpec
    from jax.experimental.shard_map import shard_map
    import concourse.mybir as mybir
    from concourse import bass2jax

    bass2jax.install_neuronx_cc_hook()
    n_cores = len(in_maps)

    partition_name = nc.partition_id_tensor.name if nc.partition_id_tensor else None
    in_names, out_names, out_avals, zero_outs = [], [], [], []
    for alloc in nc.m.functions[0].allocations:
        if not isinstance(alloc, mybir.MemoryLocationSet):
            continue
        name = alloc.memorylocations[0].name
        if alloc.kind == "ExternalInput":
            if name != partition_name:
                in_names.append(name)
        elif alloc.kind == "ExternalOutput":
            out_names.append(name)
            shape = tuple(alloc.tensor_shape)
            dtype = mybir.dt.np(alloc.dtype)
            out_avals.append(jax.core.ShapedArray(shape, dtype))
            zero_outs.append(_np.zeros(shape, dtype))
    n_params = len(in_names)
    all_names = in_names + out_names

    def _body(*args):
        operands = list(args)
        if partition_name is not None:
            operands.append(bass2jax.partition_id_tensor())
        outs = bass2jax._bass_exec_p.bind(
            *operands,
            out_avals=tuple(out_avals),
            in_names=tuple(all_names + ([partition_name] if partition_name else [])),
            out_names=tuple(out_names),
            lowering_input_output_aliases=(),
            sim_require_finite=True,
            sim_require_nnan=True,
            nc=nc,
        )
        return tuple(outs)

    devices = jax.devices()[:n_cores]
    mesh = Mesh(_np.asarray(devices), ("core",))
    in_specs = (PartitionSpec("core"),) * (n_params + len(out_names))
    out_specs = (PartitionSpec("core"),) * len(out_names)
    fn = jax.jit(
        shard_map(_body, mesh=mesh, in_specs=in_specs, out_specs=out_specs,
                  check_rep=False),
        keep_unused=True,
    )
    concat_in = [
        _np.concatenate([_np.asarray(in_maps[c][nm]) for c in range(n_cores)], axis=0)
        for nm in in_names
    ]
    concat_zeros = [
        _np.zeros((n_cores * z.shape[0], *z.shape[1:]), z.dtype) for z in zero_outs
    ]
    args = [jax.device_put(a) for a in concat_in + concat_zeros]
    return fn, args


# revision 18
# speedup vs baseline: 68.9606x; 68.9606x over previous
"""ALISTA (nn_ALISTA) Trainium2 kernel — data-parallel over batch on 8 NeuronCores.

Reference computation (per iteration i, 16 iterations):
    r   = d @ A.T - y                      # [B, m]
    z   = d - step_i * (r @ W)             # [B, n]
    d'  = sign(z) * max(|z| - thr_i, 0)    # soft threshold
output = all 16 iterates stacked: [16, B, n].

Shapes: B=4096, m=512, n=2048. Sharding: batch/8 -> 512 rows per core;
A, W, thr, step replicated. No cross-core communication.

Device layout (per core) keeps d TRANSPOSED as dT [n, b] so that both
matmuls need zero on-chip transposes:
  step 1: rT[m,b] = sum_n AT[n,m] * dT[n,b]   (lhsT = A.T tiles, rhs = dT)
  step 2: q[n,b]  = sum_m W[m,n] * rs[m,b]    (lhsT = W as-is,   rhs = rT - yT)
  z = dT - step*q computed as one scalar_tensor_tensor; soft-threshold via a
  ReLU pair on the Scalar engine: soft(z,t) = relu(z-t) - relu(-z-t).
Host pre-transposes A and y (cheap numpy) and un-transposes the output.

Matmul dtype (ALISTA_MM_DTYPE): bfloat16 (default; HW-measured 262 ns/MM),
float32r (368 ns/MM, ~10-bit mantissa), float32 (exact, 4x slower rows).
bf16 keeps fp32 PSUM accumulation; measured end-to-end rel err vs the fp32
reference is well inside the 2e-2 gate.
"""

import os

import numpy as np

M, N, ITERS = 512, 2048, 16
B_FULL = 4096
NCORES = 8
BL = B_FULL // NCORES  # 512 rows of y per core
KT = N // 128  # 16 n-tiles (step-1 contraction / step-2 output tiles)
MT = M // 128  # 4 m-tiles

MM_DTYPE = os.environ.get("ALISTA_MM_DTYPE", "float32r")

_CACHE = {}
LAST = {}


def build_nc(mm_dtype_str=MM_DTYPE, reps=1, timing=False):
    from concourse import bacc
    import concourse.mybir as mybir
    import concourse.tile as tile
    from contextlib import ExitStack

    f32 = mybir.dt.float32
    bf16mm_mode = mm_dtype_str == "bf16mm"  # bf16 matmuls, f32 iterate
    mmdt = mybir.dt.bfloat16 if bf16mm_mode else getattr(mybir.dt, mm_dtype_str)
    bf16_mode = mm_dtype_str == "bfloat16"
    # elementwise intermediate dtype: bf16 chain in bf16 mode (faster DVE/ACT
    # modes), f32 otherwise
    edt = mmdt if bf16_mode else f32
    # iterate (d) storage dtype
    ddt = f32 if bf16mm_mode else mmdt
    # A/W arrive from DRAM already in matmul dtype for bf16 (host converts);
    # f32r is bit-identical to f32 so the DMA just bitcasts (HW rounds on use).
    wdram_dt = mmdt if (bf16_mode or bf16mm_mode) else f32
    Relu = mybir.ActivationFunctionType.Relu
    Alu = mybir.AluOpType

    def as_f32(ap):
        if bf16_mode:
            return ap
        if bf16mm_mode:
            return ap  # already f32
        return ap.bitcast(f32)

    nc = bacc.Bacc("TRN2", target_bir_lowering=False, debug=False, num_devices=NCORES)

    at_ext = nc.dram_tensor("at", [128, KT * M], wdram_dt, kind="ExternalInput").ap()
    w_ext = nc.dram_tensor("w", [128, MT * N], wdram_dt, kind="ExternalInput").ap()
    yt_ext = nc.dram_tensor("yt", [128, MT * BL], f32, kind="ExternalInput").ap()
    cst_ext = nc.dram_tensor("cst", [128, 2 * ITERS], f32, kind="ExternalInput").ap()
    if timing:
        # identical device work, but results land in an internal DRAM buffer so
        # the jit carries no 512MB external buffers over the relay; a tiny tick
        # tensor is the only external output.
        out_ext = nc.dram_tensor("outbuf", [ITERS, N, BL], f32).ap()
        tick_ext = nc.dram_tensor("tick", [128, 1], f32, kind="ExternalOutput").ap()
    else:
        out_ext = nc.dram_tensor("out", [ITERS, N, BL], f32, kind="ExternalOutput").ap()
        tick_ext = None

    def wcast(ap):
        return ap if (bf16_mode or bf16mm_mode) else ap.bitcast(mmdt)

    with tile.TileContext(nc) as tc, ExitStack() as ctx:
        const = ctx.enter_context(tc.tile_pool(name="const", bufs=1))
        dpool = ctx.enter_context(tc.tile_pool(name="d", bufs=1))
        rspool = ctx.enter_context(tc.tile_pool(name="rs", bufs=2))
        upool = ctx.enter_context(tc.tile_pool(name="u", bufs=3))
        apool = ctx.enter_context(tc.tile_pool(name="act", bufs=3))
        opool = ctx.enter_context(tc.tile_pool(name="ostage", bufs=3))
        prpool = ctx.enter_context(tc.tile_pool(name="pr", bufs=1, space="PSUM"))
        pzpool = ctx.enter_context(tc.tile_pool(name="pz", bufs=4, space="PSUM"))

        at_sb = const.tile([128, KT * M], mmdt, name="at_sb", tag="at")
        w_sb = const.tile([128, MT * N], mmdt, name="w_sb", tag="w")
        yt_sb = const.tile([128, MT * BL], f32, name="yt_sb", tag="yt")
        cst_sb = const.tile([128, 2 * ITERS], f32, name="cst_sb", tag="cst")
        nc.sync.dma_start(cst_sb[:], cst_ext[:])
        # Interleave yt/W per m-tile so iteration 0's first matmuls start
        # early; A.T split per k-tile (only needed from iteration 1).
        for m in range(MT):
            nc.sync.dma_start(
                yt_sb[:, m * BL : (m + 1) * BL], yt_ext[:, m * BL : (m + 1) * BL]
            )
            nc.sync.dma_start(
                w_sb[:, m * N : (m + 1) * N], wcast(w_ext[:, m * N : (m + 1) * N])
            )
        for k in range(KT):
            nc.sync.dma_start(
                at_sb[:, k * M : (k + 1) * M], wcast(at_ext[:, k * M : (k + 1) * M])
            )

        # Two d buffers (ping/pong across iterations), 16 tiles each so the
        # scheduler tracks per-tile deps and can overlap iteration boundaries.
        d_sb = [
            [dpool.tile([128, BL], ddt, name=f"d{p}_{k}", tag=f"d{p}_{k}")
             for k in range(KT)]
            for p in range(2)
        ]
        db_sb = None
        if bf16mm_mode:
            db_sb = [
                [dpool.tile([128, BL], mmdt, name=f"db{p}_{k}", tag=f"db{p}_{k}")
                 for k in range(KT)]
                for p in range(2)
            ]

        for rep in range(reps):
            for it in range(ITERS):
                negthr = cst_sb[:, 2 * it : 2 * it + 1]
                negstep = cst_sb[:, 2 * it + 1 : 2 * it + 2]
                dr = d_sb[it % 2]  # previous iterate (unused when it == 0)
                dw = d_sb[(it + 1) % 2]
                dbr = db_sb[it % 2] if bf16mm_mode else dr
                dbw = db_sb[(it + 1) % 2] if bf16mm_mode else None

                # ---- step 1: rs[m] = rT_tile - yT_tile  (r = d @ A.T - y) ----
                rs = [rspool.tile([128, BL], mmdt, name=f"rs_{rep}_{it}_{m}",
                                  tag=f"rs{m}") for m in range(MT)]
                if it == 0:
                    for m in range(MT):
                        nc.vector.tensor_scalar_mul(
                            rs[m][:], yt_sb[:, m * BL : (m + 1) * BL], -1.0
                        )
                else:
                    for m in range(MT):
                        prt = prpool.tile([128, BL], f32,
                                          name=f"pr_{rep}_{it}_{m}", tag=f"pr{m}")
                        for k in range(KT):
                            nc.tensor.matmul(
                                prt[:],
                                at_sb[:, k * M + m * 128 : k * M + (m + 1) * 128],
                                dbr[k][:],
                                start=(k == 0),
                                stop=(k == KT - 1),
                            )
                        nc.vector.tensor_sub(
                            rs[m][:], prt[:], yt_sb[:, m * BL : (m + 1) * BL]
                        )

                # ---- step 2 + soft threshold, per output n-tile ----
                for n in range(KT):
                    pzt = pzpool.tile([128, BL], f32, name=f"pz_{rep}_{it}_{n}",
                                      tag="pz")
                    for m in range(MT):
                        nc.tensor.matmul(
                            pzt[:],
                            w_sb[:, m * N + n * 128 : m * N + (n + 1) * 128],
                            rs[m][:],
                            start=(m == 0),
                            stop=(m == MT - 1),
                        )
                    # z = d - step*q  == (q * (-step)) + d
                    u = upool.tile([128, BL], edt, name=f"u_{rep}_{it}_{n}", tag="u")
                    if it == 0:
                        nc.vector.tensor_scalar_mul(u[:], pzt[:], negstep)
                    else:
                        nc.vector.scalar_tensor_tensor(
                            u[:], pzt[:], negstep, as_f32(dr[n][:]),
                            op0=Alu.mult, op1=Alu.add
                        )
                    # soft(z, t) = relu(z - t) - relu(-z - t)
                    a1 = apool.tile([128, BL], edt, name=f"a1_{rep}_{it}_{n}", tag="a1")
                    a2 = apool.tile([128, BL], edt, name=f"a2_{rep}_{it}_{n}", tag="a2")
                    nc.scalar.activation(a1[:], u[:], Relu, bias=negthr, scale=1.0)
                    nc.scalar.activation(a2[:], u[:], Relu, bias=negthr, scale=-1.0)
                    nc.vector.tensor_sub(dw[n][:], a1[:], a2[:])
                    if bf16mm_mode:
                        nc.scalar.copy(dbw[n][:], dw[n][:])
                    if bf16_mode:
                        # output must be f32; stage through a ScalarE copy
                        ot = opool.tile([128, BL], f32, name=f"o_{rep}_{it}_{n}",
                                        tag="o")
                        nc.scalar.copy(ot[:], dw[n][:])
                        nc.sync.dma_start(out_ext[it, n * 128 : (n + 1) * 128, :],
                                          ot[:])
                    else:
                        nc.sync.dma_start(out_ext[it, n * 128 : (n + 1) * 128, :],
                                          as_f32(dw[n][:]))

        if timing:
            nc.sync.dma_start(tick_ext[:], cst_sb[:, 0:1])

    nc.compile()
    return nc


def _get_nc(reps=1, timing=False):
    key = (MM_DTYPE, reps, timing)
    if key not in _CACHE:
        _CACHE[key] = build_nc(MM_DTYPE, reps, timing)
    return _CACHE[key]


def make_in_maps(y, A, W, thr, step):
    y = np.asarray(y, dtype=np.float32)
    A = np.asarray(A, dtype=np.float32)
    W = np.asarray(W, dtype=np.float32)
    thr = np.asarray(thr, dtype=np.float32)
    step = np.asarray(step, dtype=np.float32)

    # [n, m] -> SBUF layout [p=128, k*M + m] with row p holding A.T[k*128+p, :]
    at_h = np.ascontiguousarray(
        A.T.reshape(KT, 128, M).transpose(1, 0, 2).reshape(128, KT * M)
    )
    w_h = np.ascontiguousarray(
        W.reshape(MT, 128, N).transpose(1, 0, 2).reshape(128, MT * N)
    )
    if MM_DTYPE in ("bfloat16", "bf16mm"):
        import ml_dtypes

        at_h = at_h.astype(ml_dtypes.bfloat16)
        w_h = w_h.astype(ml_dtypes.bfloat16)
    cst = np.zeros((128, 2 * ITERS), np.float32)
    cst[:, 0::2] = -thr[None, :]
    cst[:, 1::2] = -step[None, :]

    yT = y.T  # [m, B]
    in_maps = []
    for c in range(NCORES):
        ytc = np.ascontiguousarray(
            yT[:, c * BL : (c + 1) * BL]
            .reshape(MT, 128, BL)
            .transpose(1, 0, 2)
            .reshape(128, MT * BL)
        )
        in_maps.append({"at": at_h, "w": w_h, "yt": ytc, "cst": cst})
    return in_maps


def kernel(y, A, W, thr, step):
    from concourse.bass_utils import run_bass_kernel_spmd

    nc = _get_nc()
    in_maps = make_in_maps(y, A, W, thr, step)

    res = run_bass_kernel_spmd(nc, in_maps, list(range(NCORES)))
    LAST["exec_time_ns"] = res.exec_time_ns
    results = res.results

    # per-core out: [16, n, b_local] -> full [16, B, n]
    out = np.concatenate([r["out"].transpose(0, 2, 1) for r in results], axis=1)
    return np.ascontiguousarray(out, dtype=np.float32)


def make_exec_fn(nc, in_maps):
    """Build a re-executable jitted fn over the 8-core mesh (no donation, so
    it can be called repeatedly on resident device buffers) for timing.
    Mirrors bass2jax.run_bass_via_pjrt's multi-core path."""
    import jax
    import numpy as _np
    from jax.sharding import Mesh, PartitionSpec
    from jax.experimental.shard_map import shard_map
    import concourse.mybir as mybir
    from concourse import bass2jax

    bass2jax.install_neuronx_cc_hook()
    n_cores = len(in_maps)

    partition_name = nc.partition_id_tensor.name if nc.partition_id_tensor else None
    in_names, out_names, out_avals, zero_outs = [], [], [], []
    for alloc in nc.m.functions[0].allocations:
        if not isinstance(alloc, mybir.MemoryLocationSet):
            continue
        name = alloc.memorylocations[0].name
        if alloc.kind == "ExternalInput":
            if name != partition_name:
                in_names.append(name)
        elif alloc.kind == "ExternalOutput":
            out_names.append(name)
            shape = tuple(alloc.tensor_shape)
            dtype = mybir.dt.np(alloc.dtype)
            out_avals.append(jax.core.ShapedArray(shape, dtype))
            zero_outs.append(_np.zeros(shape, dtype))
    n_params = len(in_names)
    all_names = in_names + out_names

    def _body(*args):
        operands = list(args)
        if partition_name is not None:
            operands.append(bass2jax.partition_id_tensor())
        outs = bass2jax._bass_exec_p.bind(
            *operands,
            out_avals=tuple(out_avals),
            in_names=tuple(all_names + ([partition_name] if partition_name else [])),
            out_names=tuple(out_names),
            lowering_input_output_aliases=(),
            sim_require_finite=True,
            sim_require_nnan=True,
            nc=nc,
        )
        return tuple(outs)

    devices = jax.devices()[:n_cores]
    mesh = Mesh(_np.asarray(devices), ("core",))
    in_specs = (PartitionSpec("core"),) * (n_params + len(out_names))
    out_specs = (PartitionSpec("core"),) * len(out_names)
    fn = jax.jit(
        shard_map(_body, mesh=mesh, in_specs=in_specs, out_specs=out_specs,
                  check_rep=False),
        keep_unused=True,
    )
    concat_in = [
        _np.concatenate([_np.asarray(in_maps[c][nm]) for c in range(n_cores)], axis=0)
        for nm in in_names
    ]
    concat_zeros = [
        _np.zeros((n_cores * z.shape[0], *z.shape[1:]), z.dtype) for z in zero_outs
    ]
    args = [jax.device_put(a) for a in concat_in + concat_zeros]
    return fn, args


# revision 23
# speedup vs baseline: 97.9937x; 1.4210x over previous
"""ALISTA (nn_ALISTA) Trainium2 kernel — data-parallel over batch on 8 NeuronCores.

Reference computation (per iteration i, 16 iterations):
    r   = d @ A.T - y                      # [B, m]
    z   = d - step_i * (r @ W)             # [B, n]
    d'  = sign(z) * max(|z| - thr_i, 0)    # soft threshold
output = all 16 iterates stacked: [16, B, n].

Shapes: B=4096, m=512, n=2048. Sharding: batch/8 -> 512 rows per core;
A, W, thr, step replicated. No cross-core communication.

Device layout (per core) keeps d TRANSPOSED as dT [n, b] so that both
matmuls need zero on-chip transposes:
  step 1: rT[m,b] = sum_n AT[n,m] * dT[n,b]   (lhsT = A.T tiles, rhs = dT)
  step 2: q[n,b]  = sum_m W[m,n] * rs[m,b]    (lhsT = W as-is,   rhs = rT - yT)
  z = dT - step*q computed as one scalar_tensor_tensor; soft-threshold via a
  ReLU pair on the Scalar engine: soft(z,t) = relu(z-t) - relu(-z-t).
Host pre-transposes A and y (cheap numpy) and un-transposes the output.

Matmul dtype (ALISTA_MM_DTYPE): f16mm (default) runs the matmuls in fp16 with
an exact per-iteration power-of-2 rescale (soft-threshold is positively
homogeneous: soft(z,t)/s == soft(z/s, t/s)), folded for free into the ReLU
scale/bias; the iterate ships as fp16 and the host undoes the scales exactly.
fp16's 2-byte weights load ~1.4x faster than float32r's fused 4-byte path
(HW-measured) at the same ~10-bit mantissa. Fallbacks: float32r (safe,
~1.27x slower), float32 (exact, 4x slower rows). PSUM accumulation is fp32
in all modes; end-to-end rel err vs the fp32 reference: f16mm 2.9e-3,
float32r 1.5e-3 (gate 2e-2).
"""

import os

import numpy as np

M, N, ITERS = 512, 2048, 16
B_FULL = 4096
NCORES = 8
BL = B_FULL // NCORES  # 512 rows of y per core
KT = N // 128  # 16 n-tiles (step-1 contraction / step-2 output tiles)
MT = M // 128  # 4 m-tiles

MM_DTYPE = os.environ.get("ALISTA_MM_DTYPE", "f16mm")

_CACHE = {}
LAST = {}
_LAST_SIG = None  # per-iteration pow2 scale schedule (f16mm mode)


def _sigma_schedule(y, A, W, thr, step, nsub=128):
    """Per-iteration power-of-2 scales so that d_i / sig[i+1] stays well inside
    fp16 range. Estimated from a batch subsample of the exact iteration."""
    import math

    d = np.zeros((nsub, A.shape[1]), np.float32)
    sig = np.ones(ITERS + 1, np.float64)
    ys = y[:nsub]
    for i in range(ITERS):
        r = d @ A.T - ys
        z = d - step[i] * (r @ W)
        d = np.sign(z) * np.maximum(np.abs(z) - thr[i], 0.0)
        mx = max(float(np.abs(d).max()), 1e-6)
        # subsample max * 16x safety, target scaled max ~2048
        sig[i + 1] = 2.0 ** max(0, math.ceil(math.log2(mx * 16.0 / 32768.0)))
    return sig.astype(np.float64)


def build_nc(mm_dtype_str=MM_DTYPE, reps=1, timing=False, sig=None):
    from concourse import bacc
    import concourse.mybir as mybir
    import concourse.tile as tile
    from contextlib import ExitStack

    f32 = mybir.dt.float32
    bf16mm_mode = mm_dtype_str == "bf16mm"  # bf16 matmuls, f32 iterate
    f16_mode = mm_dtype_str == "f16mm"  # fp16 matmuls + per-iteration pow2 rescale
    if bf16mm_mode:
        mmdt = mybir.dt.bfloat16
    elif f16_mode:
        mmdt = mybir.dt.float16
    else:
        mmdt = getattr(mybir.dt, mm_dtype_str)
    bf16_mode = mm_dtype_str == "bfloat16"
    # elementwise intermediate dtype: bf16 chain in bf16 mode (faster DVE/ACT
    # modes), f32 otherwise
    edt = mmdt if bf16_mode else f32
    # iterate (d) storage dtype
    ddt = f32 if bf16mm_mode else mmdt
    # A/W arrive from DRAM already in matmul dtype for bf16 (host converts);
    # f32r is bit-identical to f32 so the DMA just bitcasts (HW rounds on use).
    wdram_dt = mmdt if (bf16_mode or bf16mm_mode or f16_mode) else f32
    Relu = mybir.ActivationFunctionType.Relu
    Alu = mybir.AluOpType

    ncst = 6 * ITERS if f16_mode else 2 * ITERS

    def as_f32(ap):
        if bf16_mode or bf16mm_mode or f16_mode:
            return ap
        return ap.bitcast(f32)

    nc = bacc.Bacc("TRN2", target_bir_lowering=False, debug=False, num_devices=NCORES)

    at_ext = nc.dram_tensor("at", [128, KT * M], wdram_dt, kind="ExternalInput").ap()
    w_ext = nc.dram_tensor("w", [128, MT * N], wdram_dt, kind="ExternalInput").ap()
    yt_ext = nc.dram_tensor("yt", [128, MT * BL], f32, kind="ExternalInput").ap()
    cst_ext = nc.dram_tensor("cst", [128, ncst], f32, kind="ExternalInput").ap()
    if timing:
        # identical device work, but results land in an internal DRAM buffer so
        # the jit carries no 512MB external buffers over the relay; a tiny tick
        # tensor is the only external output.
        out_ext = nc.dram_tensor("outbuf", [ITERS, N, BL],
                                 mmdt if f16_mode else f32).ap()
        tick_ext = nc.dram_tensor("tick", [128, 1], f32, kind="ExternalOutput").ap()
    else:
        out_ext = nc.dram_tensor("out", [ITERS, N, BL],
                                 mmdt if f16_mode else f32,
                                 kind="ExternalOutput").ap()
        tick_ext = None

    def wcast(ap):
        return ap if (bf16_mode or bf16mm_mode or f16_mode) else ap.bitcast(mmdt)

    with tile.TileContext(nc) as tc, ExitStack() as ctx:
        const = ctx.enter_context(tc.tile_pool(name="const", bufs=1))
        dpool = ctx.enter_context(tc.tile_pool(name="d", bufs=1))
        rspool = ctx.enter_context(tc.tile_pool(name="rs", bufs=2))
        upool = ctx.enter_context(tc.tile_pool(name="u", bufs=3))
        apool = ctx.enter_context(tc.tile_pool(name="act", bufs=3))
        opool = ctx.enter_context(tc.tile_pool(name="ostage", bufs=3))
        prpool = ctx.enter_context(tc.tile_pool(name="pr", bufs=1, space="PSUM"))
        pzpool = ctx.enter_context(tc.tile_pool(name="pz", bufs=4, space="PSUM"))

        at_sb = const.tile([128, KT * M], mmdt, name="at_sb", tag="at")
        w_sb = const.tile([128, MT * N], mmdt, name="w_sb", tag="w")
        yt_sb = const.tile([128, MT * BL], f32, name="yt_sb", tag="yt")
        cst_sb = const.tile([128, ncst], f32, name="cst_sb", tag="cst")
        nc.sync.dma_start(cst_sb[:], cst_ext[:])
        # Interleave yt/W per m-tile so iteration 0's first matmuls start
        # early; A.T split per k-tile (only needed from iteration 1).
        for m in range(MT):
            nc.sync.dma_start(
                yt_sb[:, m * BL : (m + 1) * BL], yt_ext[:, m * BL : (m + 1) * BL]
            )
            nc.sync.dma_start(
                w_sb[:, m * N : (m + 1) * N], wcast(w_ext[:, m * N : (m + 1) * N])
            )
        for k in range(KT):
            nc.sync.dma_start(
                at_sb[:, k * M : (k + 1) * M], wcast(at_ext[:, k * M : (k + 1) * M])
            )

        # Two d buffers (ping/pong across iterations), 16 tiles each so the
        # scheduler tracks per-tile deps and can overlap iteration boundaries.
        d_sb = [
            [dpool.tile([128, BL], ddt, name=f"d{p}_{k}", tag=f"d{p}_{k}")
             for k in range(KT)]
            for p in range(2)
        ]
        db_sb = None
        if bf16mm_mode:
            db_sb = [
                [dpool.tile([128, BL], mmdt, name=f"db{p}_{k}", tag=f"db{p}_{k}")
                 for k in range(KT)]
                for p in range(2)
            ]

        for rep in range(reps):
            for it in range(ITERS):
                if f16_mode:
                    # cols: [-rho*thr/sig, +rho, -rho, -step, -1/sig, pad]
                    negrhothr = cst_sb[:, 6 * it : 6 * it + 1]
                    # AP-valued activation `scale` costs ~215ns extra per op on
                    # ACT (HW-measured); rho is known at build time, so bake it.
                    rho_imm = float(sig[it] / sig[it + 1])
                    negstep = cst_sb[:, 6 * it + 3 : 6 * it + 4]
                    negc1 = cst_sb[:, 6 * it + 4 : 6 * it + 5]
                else:
                    negthr = cst_sb[:, 2 * it : 2 * it + 1]
                    negstep = cst_sb[:, 2 * it + 1 : 2 * it + 2]
                dr = d_sb[it % 2]  # previous iterate (unused when it == 0)
                dw = d_sb[(it + 1) % 2]
                dbr = db_sb[it % 2] if bf16mm_mode else dr
                dbw = db_sb[(it + 1) % 2] if bf16mm_mode else None

                # ---- step 1: rs[m] = rT_tile - yT_tile  (r = d @ A.T - y) ----
                rs = [rspool.tile([128, BL], mmdt, name=f"rs_{rep}_{it}_{m}",
                                  tag=f"rs{m}") for m in range(MT)]
                if it == 0:
                    for m in range(MT):
                        nc.vector.tensor_scalar_mul(
                            rs[m][:], yt_sb[:, m * BL : (m + 1) * BL],
                            negc1 if f16_mode else -1.0,
                        )
                else:
                    for m in range(MT):
                        prt = prpool.tile([128, BL], f32,
                                          name=f"pr_{rep}_{it}_{m}", tag=f"pr{m}")
                        for k in range(KT):
                            nc.tensor.matmul(
                                prt[:],
                                at_sb[:, k * M + m * 128 : k * M + (m + 1) * 128],
                                dbr[k][:],
                                start=(k == 0),
                                stop=(k == KT - 1),
                            )
                        if f16_mode:
                            # rs = (yt * -1/sig) + psum_r
                            nc.vector.scalar_tensor_tensor(
                                rs[m][:], yt_sb[:, m * BL : (m + 1) * BL],
                                negc1, prt[:], op0=Alu.mult, op1=Alu.add,
                            )
                        else:
                            nc.vector.tensor_sub(
                                rs[m][:], prt[:], yt_sb[:, m * BL : (m + 1) * BL]
                            )

                # ---- step 2 + soft threshold, per output n-tile ----
                for n in range(KT):
                    pzt = pzpool.tile([128, BL], f32, name=f"pz_{rep}_{it}_{n}",
                                      tag="pz")
                    for m in range(MT):
                        nc.tensor.matmul(
                            pzt[:],
                            w_sb[:, m * N + n * 128 : m * N + (n + 1) * 128],
                            rs[m][:],
                            start=(m == 0),
                            stop=(m == MT - 1),
                        )
                    # z = d - step*q  == (q * (-step)) + d
                    u = upool.tile([128, BL], edt, name=f"u_{rep}_{it}_{n}", tag="u")
                    if it == 0:
                        nc.vector.tensor_scalar_mul(u[:], pzt[:], negstep)
                    else:
                        nc.vector.scalar_tensor_tensor(
                            u[:], pzt[:], negstep, as_f32(dr[n][:]),
                            op0=Alu.mult, op1=Alu.add
                        )
                    # soft(z, t) = relu(z - t) - relu(-z - t)
                    a1 = apool.tile([128, BL], edt, name=f"a1_{rep}_{it}_{n}", tag="a1")
                    a2 = apool.tile([128, BL], edt, name=f"a2_{rep}_{it}_{n}", tag="a2")
                    if f16_mode:
                        # rho*soft(u, t~) = relu(rho*u - rho*t~) - relu(-rho*u - rho*t~)
                        nc.scalar.activation(a1[:], u[:], Relu, bias=negrhothr,
                                             scale=rho_imm)
                        nc.scalar.activation(a2[:], u[:], Relu, bias=negrhothr,
                                             scale=-rho_imm)
                    else:
                        nc.scalar.activation(a1[:], u[:], Relu, bias=negthr, scale=1.0)
                        nc.scalar.activation(a2[:], u[:], Relu, bias=negthr, scale=-1.0)
                    nc.vector.tensor_sub(dw[n][:], a1[:], a2[:])
                    if bf16mm_mode:
                        nc.scalar.copy(dbw[n][:], dw[n][:])
                    if bf16_mode:
                        # output must be f32; stage through a ScalarE copy
                        ot = opool.tile([128, BL], f32, name=f"o_{rep}_{it}_{n}",
                                        tag="o")
                        nc.scalar.copy(ot[:], dw[n][:])
                        nc.sync.dma_start(out_ext[it, n * 128 : (n + 1) * 128, :],
                                          ot[:])
                    else:
                        nc.sync.dma_start(out_ext[it, n * 128 : (n + 1) * 128, :],
                                          as_f32(dw[n][:]))

        if timing:
            nc.sync.dma_start(tick_ext[:], cst_sb[:, 0:1])

    nc.compile()
    return nc


def _get_nc(reps=1, timing=False, sig=None):
    if sig is None and MM_DTYPE == "f16mm":
        sig = _LAST_SIG if _LAST_SIG is not None else np.ones(ITERS + 1)
    sig_key = tuple(float(s) for s in sig) if sig is not None else None
    key = (MM_DTYPE, reps, timing, sig_key)
    if key not in _CACHE:
        _CACHE[key] = build_nc(MM_DTYPE, reps, timing, sig=sig)
    return _CACHE[key]


def make_in_maps(y, A, W, thr, step):
    y = np.asarray(y, dtype=np.float32)
    A = np.asarray(A, dtype=np.float32)
    W = np.asarray(W, dtype=np.float32)
    thr = np.asarray(thr, dtype=np.float32)
    step = np.asarray(step, dtype=np.float32)

    # [n, m] -> SBUF layout [p=128, k*M + m] with row p holding A.T[k*128+p, :]
    at_h = np.ascontiguousarray(
        A.T.reshape(KT, 128, M).transpose(1, 0, 2).reshape(128, KT * M)
    )
    w_h = np.ascontiguousarray(
        W.reshape(MT, 128, N).transpose(1, 0, 2).reshape(128, MT * N)
    )
    global _LAST_SIG
    if MM_DTYPE in ("bfloat16", "bf16mm"):
        import ml_dtypes

        at_h = at_h.astype(ml_dtypes.bfloat16)
        w_h = w_h.astype(ml_dtypes.bfloat16)
    if MM_DTYPE == "f16mm":
        at_h = at_h.astype(np.float16)
        w_h = w_h.astype(np.float16)
        sig = _sigma_schedule(y, A, W, thr, step)
        _LAST_SIG = sig
        cst = np.zeros((128, 6 * ITERS), np.float32)
        for i in range(ITERS):
            rho = sig[i] / sig[i + 1]
            cst[:, 6 * i + 0] = -rho * thr[i] / sig[i]
            cst[:, 6 * i + 1] = rho
            cst[:, 6 * i + 2] = -rho
            cst[:, 6 * i + 3] = -step[i]
            cst[:, 6 * i + 4] = -1.0 / sig[i]
    else:
        cst = np.zeros((128, 2 * ITERS), np.float32)
        cst[:, 0::2] = -thr[None, :]
        cst[:, 1::2] = -step[None, :]

    yT = y.T  # [m, B]
    in_maps = []
    for c in range(NCORES):
        ytc = np.ascontiguousarray(
            yT[:, c * BL : (c + 1) * BL]
            .reshape(MT, 128, BL)
            .transpose(1, 0, 2)
            .reshape(128, MT * BL)
        )
        in_maps.append({"at": at_h, "w": w_h, "yt": ytc, "cst": cst})
    return in_maps


def kernel(y, A, W, thr, step):
    from concourse.bass_utils import run_bass_kernel_spmd

    in_maps = make_in_maps(y, A, W, thr, step)
    nc = _get_nc(sig=_LAST_SIG)

    res = run_bass_kernel_spmd(nc, in_maps, list(range(NCORES)))
    LAST["exec_time_ns"] = res.exec_time_ns
    results = res.results

    # per-core out: [16, n, b_local] -> full [16, B, n]
    out = np.concatenate([r["out"].transpose(0, 2, 1) for r in results], axis=1)
    out = np.ascontiguousarray(out, dtype=np.float32)
    if MM_DTYPE == "f16mm":
        # device computed d_i / sig[i+1] in fp16; undo the exact pow2 scales
        out *= np.asarray(_LAST_SIG[1:], np.float32)[:, None, None]
    return out


def make_exec_fn(nc, in_maps):
    """Build a re-executable jitted fn over the 8-core mesh (no donation, so
    it can be called repeatedly on resident device buffers) for timing.
    Mirrors bass2jax.run_bass_via_pjrt's multi-core path."""
    import jax
    import numpy as _np
    from jax.sharding import Mesh, PartitionSpec
    from jax.experimental.shard_map import shard_map
    import concourse.mybir as mybir
    from concourse import bass2jax

    bass2jax.install_neuronx_cc_hook()
    n_cores = len(in_maps)

    partition_name = nc.partition_id_tensor.name if nc.partition_id_tensor else None
    in_names, out_names, out_avals, zero_outs = [], [], [], []
    for alloc in nc.m.functions[0].allocations:
        if not isinstance(alloc, mybir.MemoryLocationSet):
            continue
        name = alloc.memorylocations[0].name
        if alloc.kind == "ExternalInput":
            if name != partition_name:
                in_names.append(name)
        elif alloc.kind == "ExternalOutput":
            out_names.append(name)
            shape = tuple(alloc.tensor_shape)
            dtype = mybir.dt.np(alloc.dtype)
            out_avals.append(jax.core.ShapedArray(shape, dtype))
            zero_outs.append(_np.zeros(shape, dtype))
    n_params = len(in_names)
    all_names = in_names + out_names

    def _body(*args):
        operands = list(args)
        if partition_name is not None:
            operands.append(bass2jax.partition_id_tensor())
        outs = bass2jax._bass_exec_p.bind(
            *operands,
            out_avals=tuple(out_avals),
            in_names=tuple(all_names + ([partition_name] if partition_name else [])),
            out_names=tuple(out_names),
            lowering_input_output_aliases=(),
            sim_require_finite=True,
            sim_require_nnan=True,
            nc=nc,
        )
        return tuple(outs)

    devices = jax.devices()[:n_cores]
    mesh = Mesh(_np.asarray(devices), ("core",))
    in_specs = (PartitionSpec("core"),) * (n_params + len(out_names))
    out_specs = (PartitionSpec("core"),) * len(out_names)
    fn = jax.jit(
        shard_map(_body, mesh=mesh, in_specs=in_specs, out_specs=out_specs,
                  check_rep=False),
        keep_unused=True,
    )
    concat_in = [
        _np.concatenate([_np.asarray(in_maps[c][nm]) for c in range(n_cores)], axis=0)
        for nm in in_names
    ]
    concat_zeros = [
        _np.zeros((n_cores * z.shape[0], *z.shape[1:]), z.dtype) for z in zero_outs
    ]
    args = [jax.device_put(a) for a in concat_in + concat_zeros]
    return fn, args
